# revision 19
# baseline (speedup 1.0000x reference)
"""MemoryGate kernel for Trainium2 (8 NeuronCores, SPMD).

Math (per batch b):
    mp   = memory[b] @ W_mem.T                      [M, D]
    S    = hidden[b] @ mp.T / sqrt(D)               [N, M]
    A    = softmax(S, axis=-1)
    ctx  = A @ mp                                   [N, D]
    gate = sigmoid(hidden @ Wg_h.T + ctx @ Wg_c.T + b_gate)
    out  = rmsnorm(hidden + gate * ctx) * norm_w

Sharding: 8 cores = 4 batches x 2 N-halves. Each core computes mp for its
batch (duplicated across the pair) and processes N/2 = 2048 rows.

All large matmuls run in fp8 (e4m3) with perf_mode=DoubleRow (K=256 per
instruction at 0.5 cycles per output column). Transposed copies that feed
fp8 matmuls are ALSO DoubleRow matmuls against a two-plane identity
(ident2), which runs at half the cost of a PE transpose:
    out[x, f(0..255)] = sum_{p,pl} in[p, pl, x] * ident2[p, pl, f],
    ident2[p, 0, f] = (f == p), ident2[p, 1, f] = (f == 128 + p).

Stage A: mpT [D, M] via fp8 DR matmul from WmT/memT; mp [M, D] derived
from mpT with ident2 DR transposes. Both stay resident in SBUF as fp8.

Gate weights (prepacked WghT/WgcT) are loaded ONCE into SBUF and stay
resident (8MB) instead of being re-streamed per block.

Per N-block (NB=512, 4 blocks): scores (DR) -> exp into fp8 attn + f32
row sums -> attnT via ident2 DR -> ctx (DR) normalized on the PSUM->SBUF
copy -> PE-transpose ctx -> ctxT fp8 -> gate G1+G2 (DR, resident
weights) + b_gate via a K=2 fp8 DR matmul -> sigmoid per 512-chunk,
immediately fused with ctx in place (fo = gate*ctx overwrites ctxt) ->
hidden added by an accumulate-DMA -> sum-of-squares via one DVE
tensor_tensor_reduce -> rstd -> bf16 out.

PSUM->SBUF copies alternate between ACT, DVE and Pool so no single
engine shadows the PE. The epilogue is all-bf16 (DVE 2x/4x modes).
"""

import math
import os
import sys

for _p in ("/opt/trn_rl_repo", "/root/.axon_site/_ro/trn_rl_repo"):
    if os.path.isdir(_p) and _p not in sys.path:
        sys.path.append(_p)

import numpy as np

P = 128


def build_program(BN=2048, M=2048, D=2048, E=1024, NB=512, FC=512,
                  nw_ones=True, stop_after=None, timing_mode=False):
    """Build the per-core Bass program. All shapes must divide evenly."""
    import concourse.tile as tile
    from concourse import bacc, mybir

    f32 = mybir.dt.float32
    bf16 = mybir.dt.bfloat16
    AF = mybir.ActivationFunctionType
    ALU = mybir.AluOpType
    AX = mybir.AxisListType

    kE, kD, mT, nT = E // P, D // P, M // P, NB // P
    NBLK = BN // NB
    mFC, dFC = M // FC, D // FC
    SCALE = 1.0 / math.sqrt(D)
    EPS = 1e-6

    nc = bacc.Bacc("TRN2", target_bir_lowering=False, debug=False)
    fp8 = mybir.dt.float8e4

    if not timing_mode:
        hT8d = nc.dram_tensor("hiddenT8", [NBLK * P, kD * NB], fp8, kind="ExternalInput")
        hidb = nc.dram_tensor("hiddenb", [BN, D], bf16, kind="ExternalInput")
        memT = nc.dram_tensor("memT", [E, M], fp8, kind="ExternalInput")
        WmT = nc.dram_tensor("WmT", [E, D], fp8, kind="ExternalInput")
        WghT = nc.dram_tensor("WghT", [dFC * P, kD * FC], fp8, kind="ExternalInput")
        WgcT = nc.dram_tensor("WgcT", [dFC * P, kD * FC], fp8, kind="ExternalInput")
        bias2d = nc.dram_tensor("bias2", [1, 2 * D], fp8, kind="ExternalInput")
        ones2d = nc.dram_tensor("ones2", [1, 2 * P], fp8, kind="ExternalInput")
        onesmd = nc.dram_tensor("onesm", [P, 2], fp8, kind="ExternalInput")
        id2d = nc.dram_tensor("ident2", [P, 2 * 2 * P], fp8, kind="ExternalInput")
        idd = nc.dram_tensor("ident", [P, P], bf16, kind="ExternalInput")
        nw = nc.dram_tensor("norm_w", [1, D], f32, kind="ExternalInput")
        out = nc.dram_tensor("out", [BN, D], bf16, kind="ExternalOutput")
    else:
        seed = nc.dram_tensor("seed", [1, P], f32, kind="ExternalInput")
        outt = nc.dram_tensor("outt", [1, P], bf16, kind="ExternalOutput")

    with tile.TileContext(nc) as tc:
        with (
            tc.tile_pool(name="dram", bufs=1, space="DRAM") as dpool,
            tc.tile_pool(name="const", bufs=1) as const,
        ):
            if timing_mode:
                hT8d = dpool.tile([NBLK * P, kD * NB], fp8, tag="t_hT8", name="t_hT8")
                hidb = dpool.tile([BN, D], bf16, tag="t_hid", name="t_hid")
                memT = dpool.tile([E, M], fp8, tag="t_memT", name="t_memT")
                WmT = dpool.tile([E, D], fp8, tag="t_WmT", name="t_WmT")
                WghT = dpool.tile([dFC * P, kD * FC], fp8, tag="t_Wgh", name="t_Wgh")
                WgcT = dpool.tile([dFC * P, kD * FC], fp8, tag="t_Wgc", name="t_Wgc")
                bias2d = dpool.tile([1, 2 * D], fp8, tag="t_bg", name="t_bg")
                ones2d = dpool.tile([1, 2 * P], fp8, tag="t_on", name="t_on")
                onesmd = dpool.tile([P, 2], fp8, tag="t_onm", name="t_onm")
                id2d = dpool.tile([P, 2 * 2 * P], fp8, tag="t_id2", name="t_id2")
                idd = dpool.tile([P, P], bf16, tag="t_idd", name="t_idd")
                nw = dpool.tile([1, D], f32, tag="t_nw", name="t_nw")
                out = dpool.tile([BN, D], bf16, tag="t_out", name="t_out")
                with tc.tile_pool(name="init", bufs=1) as ipool:
                    zb = ipool.tile([P, 2 * D], bf16, tag="zb", name="zb")
                    nc.vector.memset(zb, 0.0)
                    zf = ipool.tile([P, D], f32, tag="zf", name="zf")
                    nc.vector.memset(zf, 0.0)
                    z8 = ipool.tile([P, kD * NB], fp8, tag="z8", name="z8")
                    nc.vector.memset(z8, 0.0)
                    for t, rows, cols in ((hT8d, NBLK * P, kD * NB),
                                          (WghT, dFC * P, kD * FC),
                                          (WgcT, dFC * P, kD * FC),
                                          (memT, E, M), (WmT, E, D)):
                        for r in range(0, rows, P):
                            nc.sync.dma_start(t[r:r + P, :], z8[:, :cols])
                    for r in range(0, BN, P):
                        nc.sync.dma_start(hidb[r:r + P, :], zb[:, :D])
                    nc.sync.dma_start(bias2d[:], z8[:1, :2 * D])
                    nc.sync.dma_start(ones2d[:], z8[:1, :2 * P])
                    nc.sync.dma_start(onesmd[:], z8[:, :2])
                    nc.sync.dma_start(id2d[:], z8[:, :4 * P])
                    nc.sync.dma_start(idd[:], zb[:, :P])
                    nc.sync.dma_start(nw[:], zf[:1, :D])
                    sd = ipool.tile([1, P], f32, tag="sd", name="sd")
                    nc.sync.dma_start(sd, seed[:])
                    nc.sync.dma_start(nw[:1, :P], sd)

            # ---------------- constants ----------------
            # (only ident2 is DMA'd up front; the rest are issued after the
            # stage-A input streams so the first matmuls start ASAP)
            ident2 = const.tile([P, 2, 2 * P], fp8, tag="id2", name="id2_sb")
            nc.sync.dma_start(ident2, id2d[:].rearrange("p (c x) -> p c x", c=2))
            ident = const.tile([P, P], bf16, tag="ident", name="ident_sb")
            ones2 = const.tile([1, 2, P], fp8, tag="on2", name="on2_sb")
            onesm = const.tile([P, 2, 1], fp8, tag="onm", name="onm_sb")
            bias2 = const.tile([1, 2, D], fp8, tag="bi2", name="bi2_sb")
            nb1 = const.tile([P, 1], f32, tag="nb1", name="nb1_sb")
            nc.vector.memset(nb1, -2.0)
            eps_t = const.tile([P, 1], f32, tag="eps", name="eps_sb")
            nc.vector.memset(eps_t, EPS)
            if not nw_ones:
                nw_sb = const.tile([P, D], f32, tag="nw", name="nw_sb")
                nc.gpsimd.dma_start(nw_sb, nw[:].partition_broadcast(P))

            # ---------------- resident tensors ----------------
            hold_cm = tc.tile_pool(name="hold", bufs=1)
            hold = hold_cm.__enter__()
            mpT8_sb = hold.tile([P, kD, M], fp8, tag="mpT8", name="mpT8_sb")
            mp8_sb = hold.tile([P, mT, D], fp8, tag="mp8", name="mp8_sb")
            Wgh_sb = hold.tile([P, dFC, kD, FC], fp8, tag="Wgh", name="Wgh_sb")
            Wgc_sb = hold.tile([P, dFC, kD, FC], fp8, tag="Wgc", name="Wgc_sb")

            # PSUM->SBUF copies may only run on ACT/DVE (GPSIMD has no
            # PSUM access on real hardware)
            def copy2(idx, dst, src):
                if idx % 2 == 0:
                    nc.scalar.copy(dst, src)
                else:
                    nc.vector.tensor_copy(dst, src)

            # hT8 tiles live in their own pool so block 0's load can be
            # issued during stage A (before the stage-A pool-close barrier)
            ht_cm = tc.tile_pool(name="ht", bufs=1)
            htp = ht_cm.__enter__()

            def hT8_load(blk):
                t = htp.tile([P, kD, NB], fp8, tag="hT8", bufs=2,
                             name=f"hT8_{blk}")
                nc.sync.dma_start(
                    t,
                    hT8d[blk * P:(blk + 1) * P, :]
                    .rearrange("p (k n) -> p k n", k=kD))
                return t

            # ---------------- Stage A ----------------
            # A1: mpT[d, m] = sum_e WmT[e, d] * memT[e, m]  (fp8 DR)
            # A2: mp[m, d] from mpT via ident2 DR transposes
            with (
                tc.tile_pool(name="a_in", bufs=1) as a_in,
                tc.tile_pool(name="ps", bufs=8, space="PSUM") as ps_pool,
            ):
                memT_sb = a_in.tile([P, kE, M], fp8, tag="memT", name="memT_sb")
                WmT_sb = a_in.tile([P, kE, D], fp8, tag="WmT", name="WmT_sb")
                for k in range(kE):
                    nc.scalar.dma_start(memT_sb[:, k, :], memT[k * P:(k + 1) * P, :])
                    nc.sync.dma_start(WmT_sb[:, k, :], WmT[k * P:(k + 1) * P, :])
                hT8_tiles = {0: hT8_load(0)}
                nc.scalar.dma_start(ident, idd[:])
                nc.scalar.dma_start(ones2,
                                    ones2d[:].rearrange("o (c p) -> o c p", c=2))
                nc.scalar.dma_start(onesm,
                                    onesmd[:].rearrange("p (c o) -> p c o", c=2))
                nc.scalar.dma_start(bias2,
                                    bias2d[:].rearrange("o (c d) -> o c d", c=2))

                cidx = 0
                for j in range(kD // 2):          # d-plane pairs
                    for dp in (2 * j, 2 * j + 1):
                        for mc in range(mFC):
                            ps = ps_pool.tile([P, FC], f32, tag="ps",
                                              name=f"a1ps{dp}_{mc}")
                            for kj in range(kE // 2):
                                nc.tensor.matmul(
                                    ps,
                                    WmT_sb[:, 2 * kj:2 * kj + 2, dp * P:(dp + 1) * P],
                                    memT_sb[:, 2 * kj:2 * kj + 2, mc * FC:(mc + 1) * FC],
                                    start=(kj == 0), stop=(kj == kE // 2 - 1),
                                    perf_mode=mybir.MatmulPerfMode.DoubleRow,
                                )
                            copy2(cidx, mpT8_sb[:, dp, mc * FC:(mc + 1) * FC], ps)
                            cidx += 1
                    # transpose the freshly finished d-pair into mp8
                    for t2 in range(mT // 2):
                        tp = ps_pool.tile([P, FC], f32, tag="ps",
                                          name=f"a2ps{j}_{t2}")
                        for h in range(2):
                            mt = 2 * t2 + h
                            nc.tensor.matmul(
                                tp[:, h * 2 * P:(h + 1) * 2 * P],
                                mpT8_sb[:, 2 * j:2 * j + 2, mt * P:(mt + 1) * P],
                                ident2,
                                start=True, stop=True,
                                perf_mode=mybir.MatmulPerfMode.DoubleRow,
                            )
                        copy2(cidx, mp8_sb[:, 2 * t2:2 * t2 + 2,
                                           j * 2 * P:(j + 1) * 2 * P],
                              tp.rearrange("p (a b) -> p a b", a=2))
                        cidx += 1

            # ---------------- Stage B ----------------
            with (
                tc.tile_pool(name="b_big", bufs=1) as bb,
                tc.tile_pool(name="b_sm", bufs=2) as sm,
                tc.tile_pool(name="ps", bufs=8, space="PSUM") as bps,
            ):
                # resident gate weights: load once (after the block-0 hT8 so
                # scores aren't stuck behind 8MB of weight DMA)
                for dc in range(dFC):
                    nc.sync.dma_start(
                        Wgh_sb[:, dc, :, :],
                        WghT[dc * P:(dc + 1) * P, :]
                        .rearrange("p (t f) -> p t f", t=kD))
                for dc in range(dFC):
                    nc.sync.dma_start(
                        Wgc_sb[:, dc, :, :],
                        WgcT[dc * P:(dc + 1) * P, :]
                        .rearrange("p (t f) -> p t f", t=kD))

                for blk in range(NBLK if stop_after != "A" else 0):
                    n0 = blk * NB
                    hT8_sb = hT8_tiles.pop(blk)

                    # ---- scoresT -> attnT = exp(sT/sqrt(D) - 2) fp8 directly
                    attnT = bb.tile([P, mT, NB], fp8, tag="attnT8", bufs=1,
                                    name=f"attnT{blk}")
                    rs = sm.tile([P, nT], f32, tag="rs", name=f"rs{blk}")
                    for mp_ in range(mT):
                        ps = bps.tile([P, NB], f32, tag="ps",
                                      name=f"sc{blk}_{mp_}")
                        for kj in range(kD // 2):
                            nc.tensor.matmul(
                                ps,
                                mpT8_sb[:, 2 * kj:2 * kj + 2, mp_ * P:(mp_ + 1) * P],
                                hT8_sb[:, 2 * kj:2 * kj + 2, :],
                                start=(kj == 0), stop=(kj == kD // 2 - 1),
                                perf_mode=mybir.MatmulPerfMode.DoubleRow)
                        nc.scalar.activation(
                            attnT[:, mp_, :], ps, AF.Exp, scale=SCALE, bias=nb1)

                    # prefetch next block's hiddenT while PE chews on this one
                    if blk + 1 < NBLK:
                        hT8_tiles[blk + 1] = hT8_load(blk + 1)

                    if stop_after == "scores":
                        continue
                    # ---- ctx = attn @ mp (dc=0), then softmax denominators as
                    # F=1 DR matmuls (lands directly in [n-part, 1]), then the
                    # remaining ctx chunks; normalization rides the copies.
                    ctxt = bb.tile([P, nT, D], bf16, tag="ctx", name=f"ctx{blk}")
                    ctx8 = bb.tile([P, nT, D], fp8, tag="ctx8", name=f"c8{blk}")
                    sums_ps = None
                    for dc in range(dFC):
                        pss = [bps.tile([P, FC], f32, tag="ps",
                                        name=f"cx{blk}_{dc}_{i}") for i in range(nT)]
                        for mj in range(mT // 2):
                            for i in range(nT):
                                nc.tensor.matmul(
                                    pss[i],
                                    attnT[:, 2 * mj:2 * mj + 2, i * P:(i + 1) * P],
                                    mp8_sb[:, 2 * mj:2 * mj + 2,
                                           dc * FC:(dc + 1) * FC],
                                    start=(mj == 0), stop=(mj == mT // 2 - 1),
                                    perf_mode=mybir.MatmulPerfMode.DoubleRow)
                        if dc == 0:
                            sums_ps = bps.tile([P, nT], f32, tag="ps",
                                               name=f"sm{blk}")
                            for i in range(nT):
                                for mj in range(mT // 2):
                                    nc.tensor.matmul(
                                        sums_ps[:, i:i + 1],
                                        attnT[:, 2 * mj:2 * mj + 2,
                                              i * P:(i + 1) * P],
                                        onesm,
                                        start=(mj == 0), stop=(mj == mT // 2 - 1),
                                        perf_mode=mybir.MatmulPerfMode.DoubleRow)
                            nc.vector.reciprocal(rs, sums_ps)
                        for i in range(nT):
                            if i % 2 == 0:
                                nc.vector.tensor_scalar_mul(
                                    ctxt[:, i, dc * FC:(dc + 1) * FC],
                                    pss[i], rs[:, i:i + 1])
                            else:
                                nc.scalar.mul(
                                    ctxt[:, i, dc * FC:(dc + 1) * FC],
                                    pss[i], rs[:, i:i + 1])
                            nc.gpsimd.tensor_copy(
                                ctx8[:, i, dc * FC:(dc + 1) * FC],
                                ctxt[:, i, dc * FC:(dc + 1) * FC])

                    if stop_after == "ctx":
                        continue
                    # ---- ctxT via ident2 DR (fp8, unnormalized is fine: the
                    # per-row 1/sum rides... no -- ctx8 is unnormalized, so
                    # fold rs into the gate matmul? No: ctx8 IS the raw PSUM)
                    ctxT = bb.tile([P, kD, NB], fp8, tag="attnT8", bufs=1,
                                   name=f"ctxT{blk}")
                    for dt_ in range(kD):
                        tp = bps.tile([P, NB], f32, tag="ps",
                                      name=f"tpc{blk}_{dt_}")
                        for i2 in range(nT // 2):
                            nc.tensor.matmul(
                                tp[:, i2 * 2 * P:(i2 + 1) * 2 * P],
                                ctx8[:, 2 * i2:2 * i2 + 2,
                                     dt_ * P:(dt_ + 1) * P],
                                ident2,
                                start=True, stop=True,
                                perf_mode=mybir.MatmulPerfMode.DoubleRow)
                        copy2(dt_, ctxT[:, dt_, :], tp)

                    if stop_after == "ctxT":
                        continue
                    # ---- gate chunks + fused epilogue
                    ssq = sm.tile([P, nT], f32, tag="ssq", name=f"ssq{blk}")
                    rstd = sm.tile([P, nT], f32, tag="rstd", name=f"rstd{blk}")
                    obs = [sm.tile([P, D], bf16, tag="ob", bufs=3,
                                   name=f"ob{blk}_{i}") for i in range(nT)]
                    for dc in range(dFC):
                        pss = [bps.tile([P, FC], f32, tag="ps",
                                        name=f"gt{blk}_{dc}_{i}") for i in range(nT)]
                        for kj in range(kD // 2):
                            for i in range(nT):
                                nc.tensor.matmul(
                                    pss[i],
                                    hT8_sb[:, 2 * kj:2 * kj + 2, i * P:(i + 1) * P],
                                    Wgh_sb[:, dc, 2 * kj:2 * kj + 2, :],
                                    start=(kj == 0), stop=False,
                                    perf_mode=mybir.MatmulPerfMode.DoubleRow)
                        for i in range(nT):
                            nc.tensor.matmul(
                                pss[i], ones2,
                                bias2[:, :, dc * FC:(dc + 1) * FC],
                                start=False, stop=False,
                                perf_mode=mybir.MatmulPerfMode.DoubleRow)
                        for kj in range(kD // 2):
                            for i in range(nT):
                                nc.tensor.matmul(
                                    pss[i],
                                    ctxT[:, 2 * kj:2 * kj + 2, i * P:(i + 1) * P],
                                    Wgc_sb[:, dc, 2 * kj:2 * kj + 2, :],
                                    start=False, stop=(kj == kD // 2 - 1),
                                    perf_mode=mybir.MatmulPerfMode.DoubleRow)
                        for i in range(nT):
                            g = sm.tile([P, FC], bf16, tag="g", bufs=8,
                                        name=f"g{blk}_{dc}_{i}")
                            nc.scalar.activation(g, pss[i], AF.Sigmoid)
                            # fo = gate * ctx, in place over ctxt
                            nc.vector.tensor_mul(
                                ctxt[:, i, dc * FC:(dc + 1) * FC],
                                g, ctxt[:, i, dc * FC:(dc + 1) * FC])
                            # residual: accumulate hidden onto fo via DMA
                            nc.gpsimd.dma_start(
                                ctxt[:, i, dc * FC:(dc + 1) * FC],
                                hidb[n0 + i * P: n0 + (i + 1) * P,
                                     dc * FC:(dc + 1) * FC],
                                accum_op=ALU.add)
                            # running sum of squares for the rmsnorm
                            sqs = sm.tile([P, FC], bf16, tag="sqs", bufs=2,
                                          name=f"sq{blk}_{dc}_{i}")
                            nc.vector.tensor_tensor_reduce(
                                out=sqs,
                                in0=ctxt[:, i, dc * FC:(dc + 1) * FC],
                                in1=ctxt[:, i, dc * FC:(dc + 1) * FC],
                                scale=1.0,
                                scalar=(0.0 if dc == 0 else ssq[:, i:i + 1]),
                                op0=ALU.mult, op1=ALU.add,
                                accum_out=ssq[:, i:i + 1])

                    if stop_after == "gate":
                        continue
                    # ---- rmsnorm + writeout (one sqrt per block: avoids
                    # ACT table ping-pong with the next block's exps)
                    nc.scalar.activation(rstd, ssq, AF.Sqrt,
                                         bias=eps_t, scale=1.0 / D)
                    nc.vector.reciprocal(rstd, rstd)
                    for i in range(nT):
                        nc.vector.tensor_scalar_mul(obs[i], ctxt[:, i, :],
                                                    rstd[:, i:i + 1])
                        if not nw_ones:
                            nc.gpsimd.tensor_mul(obs[i], obs[i], nw_sb)
                        orng = out[n0 + i * P: n0 + (i + 1) * P, :]
                        if i % 2 == 0:
                            nc.sync.dma_start(orng, obs[i])
                        else:
                            nc.gpsimd.dma_start(orng, obs[i])

            ht_cm.__exit__(None, None, None)
            hold_cm.__exit__(None, None, None)

            if timing_mode:
                with tc.tile_pool(name="fin", bufs=1) as fin:
                    ft = fin.tile([1, P], bf16, tag="ft", name="ft")
                    nc.sync.dma_start(ft, out[BN - 1:BN, :P])
                    nc.sync.dma_start(outt[:], ft)

    nc.compile()
    return nc


_PROG_CACHE = {}


def _get_program(key, **kw):
    if key not in _PROG_CACHE:
        _PROG_CACHE[key] = build_program(**kw)
    return _PROG_CACHE[key]


def kernel(hidden_states, memory, W_mem, W_gate, b_gate, norm_w):
    from concourse.bass_utils import run_bass_kernel_spmd

    B, N, D = hidden_states.shape
    _, M, E = memory.shape
    NC = 8
    H = NC // B                      # N-splits per batch (2)
    BN = N // H                      # rows per core (2048)

    nw_ones = bool(np.all(np.asarray(norm_w) == 1.0))
    prog = _get_program(("full", BN, M, D, E, nw_ones),
                        BN=BN, M=M, D=D, E=E, nw_ones=nw_ones)

    import ml_dtypes
    f32 = np.float32
    bf16 = ml_dtypes.bfloat16
    fp8 = ml_dtypes.float8_e4m3
    WmT = np.ascontiguousarray(W_mem.T).astype(fp8)

    def _pack(wt, FC=512, NP=128):
        # [Dk, F] -> [(F/FC)*P rows, (Dk/P)*FC cols]: row fc*P+p, col t*FC+f
        Dd, Ff = wt.shape
        kT, fC = Dd // NP, Ff // FC
        return np.ascontiguousarray(
            wt.reshape(kT, NP, fC, FC).transpose(2, 1, 0, 3)
            .reshape(fC * NP, kT * FC))

    WghT = _pack(np.ascontiguousarray(W_gate[:, :D].T)).astype(fp8)
    WgcT = _pack(np.ascontiguousarray(W_gate[:, D:].T)).astype(fp8)
    bias2 = np.zeros((1, 2 * D), dtype=fp8)
    bias2[0, 0::2] = 0  # layout: [1, 2, D] planes contiguous
    b2 = np.zeros((2, D), dtype=np.float32)
    b2[0, :] = np.asarray(b_gate, dtype=np.float32)
    bias2 = np.ascontiguousarray(b2.reshape(1, 2 * D)).astype(fp8)
    ones2 = np.zeros((2, P), dtype=np.float32)
    ones2[0, :] = 1.0
    ones2 = np.ascontiguousarray(ones2.reshape(1, 2 * P)).astype(fp8)
    id2 = np.zeros((P, 2, 2 * P), dtype=np.float32)
    for p in range(P):
        id2[p, 0, p] = 1.0
        id2[p, 1, P + p] = 1.0
    id2 = np.ascontiguousarray(id2.reshape(P, 4 * P)).astype(fp8)
    ident = np.eye(P, dtype=np.float32).astype(bf16)
    nw = np.ascontiguousarray(norm_w[None, :], dtype=f32)

    in_maps = []
    for c in range(NC):
        b, h = c // H, c % H
        hs = hidden_states[b, h * BN:(h + 1) * BN, :]
        hsT = np.ascontiguousarray(hs.T)
        in_maps.append({
            "hiddenT8": _pack(hsT, FC=BN // 4).astype(fp8),
            "hiddenb": np.ascontiguousarray(hs).astype(bf16),
            "memT": np.ascontiguousarray(memory[b].T).astype(fp8),
            "WmT": WmT, "WghT": WghT, "WgcT": WgcT,
            "bias2": bias2, "ones2": ones2,
            "onesm": np.ones((P, 2), dtype=fp8),
            "ident2": id2, "ident": ident,
            "norm_w": nw,
        })

    res = run_bass_kernel_spmd(prog, in_maps, core_ids=list(range(NC)))
    out = np.empty((B, N, D), dtype=f32)
    for c in range(NC):
        b, h = c // H, c % H
        out[b, h * BN:(h + 1) * BN, :] = res.results[c]["out"].astype(f32)
    return out
